# revision 13
# baseline (speedup 1.0000x reference)
"""Trainium2 Bass kernel for nn_JanusModel (sparse_attention, GQA, two mask groups).

Sharding: core c in [0,8) handles batch b=c//4 and query-row block q0=(c%4)*512.
Each core computes all 16 heads for its 512 query rows -> disjoint output slices,
no collectives. All heavy operands are laid out on host (transposes/permutes only).

On-device math per core (scores kept transposed [sk, sq]):
  qT/kT/v projections (fp32r matmuls, q/k cast to bf16), scores.T = K @ qT/8
  (row-tiled head pairs, bf16), P = exp(scores) * exp(maskT) (ACT exp + DVE
  bf16 mul), AV col-tiled head pairs with a ones-column appended to V so row 64
  of each AV psum accumulates the softmax denominator for free. One batched
  reciprocal over all 16 rowsum rows, broadcast to 128 partitions via a tiny
  selector matmul, divide, then the output projection.
AV matmuls for group g are emitted after the score matmuls of group g+1 so the
PE never stalls behind the ACT->DVE chain of its own group.
"""

import os
import sys

import numpy as np

for _p in ("/opt/trn_rl_repo",):
    if os.path.isdir(_p) and _p not in sys.path:
        sys.path.insert(0, _p)

import concourse.bass as bass
import concourse.tile as tile
from concourse import bacc, mybir
from concourse.bass_utils import run_bass_kernel_spmd

B, S, D = 2, 2048, 1024
H, KVH, HD = 16, 4, 64
NCORES = 8
SQ = S // 4  # 512 query rows per core
P = 128
NKT = S // P  # 16 key tiles

# Head pairs: (a, b) share a kT tile; a uses kv head 2*(j//4), b uses +1.
PAIRS = [(0, 4), (1, 5), (2, 6), (3, 7), (8, 12), (9, 13), (10, 14), (11, 15)]

f32 = mybir.dt.float32
bf16 = mybir.dt.bfloat16
f32r = mybir.dt.float32r
EXP = mybir.ActivationFunctionType.Exp

_CACHE = {}


def _r(ap):
    return ap.bitcast(f32r)


def _body(tc, xT, wqT, wkT, wvT, woT, mT, esel, out):
    nc = tc.nc
    xT_r = xT.rearrange("(c p) s -> c p s", p=P)        # [8,128,2048]
    wqT_r = wqT.rearrange("(c p) f -> c p f", p=P)      # [8,128,1024]
    wkT_r = wkT.rearrange("(c p) f -> c p f", p=P)      # [8,128,256]
    wvT_r = wvT.rearrange("(c p) f -> c p f", p=P)      # [8,128,256]
    woT_r = woT.rearrange("(c p) d -> c p d", p=P)      # [8,128,1024]
    mT_r = mT.rearrange("m (c p) q -> m p c q", p=P)    # [2,128,16,512]
    out_r = out.rearrange("(t p) d -> t p d", p=P)      # [4,128,1024]

    persist = tc.alloc_tile_pool(name="persist", bufs=1)
    qT_sb = persist.tile([P, 8, SQ], bf16, name="qT_sb")      # pair j: a rows 0:64, b rows 64:128
    kT_sb = persist.tile([P, 2, S], bf16, name="kT_sb")       # tile jt: kv 2jt rows 0:64, kv 2jt+1 rows 64:128
    v_sb = persist.tile([P, NKT, KVH, HD + 1], bf16, name="v_sb")  # col 64 of each kv head = 1.0
    expm_sb = persist.tile([P, 2, NKT, SQ], bf16, name="expm_sb")
    attnT_sb = persist.tile([P, 8, SQ], f32r, name="attnT_sb")
    # rowsum staging: DVE writes must start at partition 0/32/64/96, so the 16
    # denominator rows (index r = 2j+half) land at partition 32*(r//4), column
    # slot r%4; one SBUF->SBUF scatter DMA later packs them to 16 partitions.
    rstage_sb = persist.tile([P, 4, SQ], f32, name="rstage_sb")

    nc.vector.memset(v_sb[:, :, :, HD:HD + 1], 1.0)

    # ---------------- phase A: masks exp + load x/w + projections ----------------
    with tc.tile_pool(name="ml", bufs=2) as mlp, \
         tc.tile_pool(name="xw", bufs=1) as xw, \
         tc.tile_pool(name="pps", bufs=4, space="PSUM") as pps:
        # mask DMAs ride the sync queue; exps fill the ACT engine while the
        # projections below stream on the gpsimd queue / PE.  bufs=3 bounds the
        # prefetch depth so mask traffic doesn't starve the x/w loads.
        for m in range(2):
            for tg in range(8):
                ml = mlp.tile([P, 2, SQ], f32, tag="ml", name=f"ml{m}{tg}")
                nc.sync.dma_start(out=ml, in_=mT_r[m, :, 2 * tg:2 * tg + 2, :])
                nc.scalar.activation(
                    out=expm_sb[:, m, 2 * tg:2 * tg + 2, :], in_=ml, func=EXP)

        x_sb = xw.tile([P, 8, S], f32r, name="x_sb")
        wq_sb = xw.tile([P, 8, H * HD], f32r, name="wq_sb")
        wk_sb = xw.tile([P, 8, KVH * HD], f32r, name="wk_sb")
        wv_sb = xw.tile([P, 8, KVH * HD], f32r, name="wv_sb")
        for c in range(8):
            nc.gpsimd.dma_start(out=x_sb[:, c, :], in_=xT_r[c])
            nc.gpsimd.dma_start(out=wq_sb[:, c, :], in_=wqT_r[c])
            nc.gpsimd.dma_start(out=wk_sb[:, c, :], in_=wkT_r[c])
        for c in range(8):
            nc.gpsimd.dma_start(out=wv_sb[:, c, :], in_=wvT_r[c])

        def qproj(j):
            # out [128 qfeat(pair j), 512]; fold 1/sqrt(HD)=1/8 scale, cast bf16
            ps = pps.tile([P, SQ], f32, tag="pq", name=f"psq{j}")
            for kc in range(8):
                nc.tensor.matmul(
                    ps, lhsT=wq_sb[:, kc, j * P:(j + 1) * P],
                    rhs=x_sb[:, kc, 0:SQ], start=(kc == 0), stop=(kc == 7))
            nc.vector.tensor_scalar_mul(qT_sb[:, j, :], ps, 0.125)

        def kproj(jt):
            for ns in range(4):
                ps = pps.tile([P, SQ], f32, tag="pq", name=f"psk{jt}{ns}")
                for kc in range(8):
                    nc.tensor.matmul(
                        ps, lhsT=wk_sb[:, kc, jt * P:(jt + 1) * P],
                        rhs=x_sb[:, kc, ns * SQ:(ns + 1) * SQ],
                        start=(kc == 0), stop=(kc == 7))
                nc.vector.tensor_copy(out=kT_sb[:, jt, ns * SQ:(ns + 1) * SQ], in_=ps)

        qproj(0)
        kproj(0)  # pair 0 consumes qT[0] + kT tile 0 first
        for j in range(1, 8):
            qproj(j)
        kproj(1)

        # v projection: natural [sk 128-tile, 4, 64] -> bf16, 65-strided slots
        for t in range(NKT):
            ps = pps.tile([P, KVH * HD], f32, tag="pv", name=f"psv{t}")
            for kc in range(8):
                nc.tensor.matmul(
                    ps, lhsT=x_sb[:, kc, t * P:(t + 1) * P],
                    rhs=wv_sb[:, kc, :], start=(kc == 0), stop=(kc == 7))
            nc.vector.tensor_copy(
                out=v_sb[:, t, :, 0:HD], in_=ps.rearrange("p (h c) -> p h c", h=KVH))

    # ---------------- phase B: attention ----------------
    with tc.tile_pool(name="psA", bufs=1, space="PSUM") as psA, \
         tc.tile_pool(name="psB", bufs=1, space="PSUM") as psB, \
         tc.tile_pool(name="avp", bufs=1, space="PSUM") as avp, \
         tc.tile_pool(name="praw", bufs=2) as praw, \
         tc.tile_pool(name="ppool", bufs=3) as ppool:
        GROUPS = [(g, min(3, NKT - 3 * g)) for g in range((NKT + 2) // 3)]
        LASTG = GROUPS[-1][0]

        def emit_av(j, g, nt, pA, pB, av_a, av_b, last):
            kv = (j // 4) * 2
            for i in range(nt):
                t = 3 * g + i
                st = (t == 0)
                sp = (t == NKT - 1)
                nc.tensor.matmul(av_a[0:HD + 1, :], lhsT=v_sb[:, t, kv, :],
                                 rhs=pA[:, i, :], start=st, stop=sp)
                nc.tensor.matmul(av_b[0:HD + 1, :], lhsT=v_sb[:, t, kv + 1, :],
                                 rhs=pB[:, i, :], start=st, stop=sp)
            if last:
                nc.vector.tensor_copy(out=attnT_sb[0:HD, j, :], in_=av_a[0:HD, :])
                nc.vector.tensor_copy(out=attnT_sb[HD:P, j, :], in_=av_b[0:HD, :])
                for half, av in ((0, av_a), (1, av_b)):
                    r = 2 * j + half
                    p0 = 32 * (r // 4)
                    nc.vector.tensor_copy(out=rstage_sb[p0:p0 + 1, r % 4, :],
                                          in_=av[HD:HD + 1, :])

        pending = None
        for j in range(8):
            jt = j // 4
            m = j // 4
            av_a = avp.tile([P, SQ], f32, tag="ava", name=f"ava{j}")
            av_b = avp.tile([P, SQ], f32, tag="avb", name=f"avb{j}")
            for g, nt in GROUPS:
                sA = psA.tile([P, 3, SQ], f32, tag="sA", name=f"sA{j}_{g}")
                sB = psB.tile([P, 3, SQ], f32, tag="sB", name=f"sB{j}_{g}")
                for i in range(nt):
                    t = 3 * g + i
                    nc.tensor.matmul(
                        sA[:, i, :], lhsT=kT_sb[0:HD, jt, t * P:(t + 1) * P],
                        rhs=qT_sb[0:HD, j, :], start=True, stop=True)
                    nc.tensor.matmul(
                        sB[:, i, :], lhsT=kT_sb[HD:P, jt, t * P:(t + 1) * P],
                        rhs=qT_sb[HD:P, j, :], start=True, stop=True)
                if pending is not None:
                    emit_av(*pending)
                    pending = None
                prA = praw.tile([P, 3, SQ], bf16, tag="prA", name=f"prA{j}_{g}")
                prB = praw.tile([P, 3, SQ], bf16, tag="prB", name=f"prB{j}_{g}")
                nc.scalar.activation(out=prA[:, 0:nt, :], in_=sA[:, 0:nt, :], func=EXP)
                nc.scalar.activation(out=prB[:, 0:nt, :], in_=sB[:, 0:nt, :], func=EXP)
                pA = ppool.tile([P, 3, SQ], bf16, tag="pA", name=f"pA{j}_{g}")
                pB = ppool.tile([P, 3, SQ], bf16, tag="pB", name=f"pB{j}_{g}")
                nc.vector.tensor_mul(pA[:, 0:nt, :], prA[:, 0:nt, :],
                                     expm_sb[:, m, 3 * g:3 * g + nt, :])
                nc.vector.tensor_mul(pB[:, 0:nt, :], prB[:, 0:nt, :],
                                     expm_sb[:, m, 3 * g:3 * g + nt, :])
                pending = (j, g, nt, pA, pB, av_a, av_b, g == LASTG)
        emit_av(*pending)

    # softmax denominators: one batched reciprocal, PE broadcast, divide
    with tc.tile_pool(name="divp", bufs=1) as divp, \
         tc.tile_pool(name="brcp", bufs=2, space="PSUM") as brcp:
        rs_sb = divp.tile([2 * 8, SQ], f32, name="rs_sb")
        rr_sb = divp.tile([2 * 8, SQ], f32r, name="rr_sb")
        esel_sb = divp.tile([2 * 8, 8, P], f32r, name="esel_sb")
        nc.gpsimd.dma_start(out=esel_sb, in_=esel)
        for s in range(4):
            nc.sync.dma_start(out=rs_sb[4 * s:4 * s + 4, :],
                              in_=rstage_sb[32 * s:32 * s + 1, :, :])
        with nc.allow_low_precision(reason="fp32r denominators feed the PE broadcast"):
            nc.vector.reciprocal(out=rr_sb, in_=rs_sb)
        for j in range(8):
            brc = brcp.tile([P, SQ], f32, tag="brc", name=f"brc{j}")
            nc.tensor.matmul(brc, lhsT=esel_sb[:, j, :], rhs=rr_sb,
                             start=True, stop=True)
            nc.vector.tensor_mul(attnT_sb[:, j, :], attnT_sb[:, j, :], brc)

    # ---------------- phase C: output projection ----------------
    with tc.tile_pool(name="wo", bufs=3) as wop, \
         tc.tile_pool(name="ops", bufs=8, space="PSUM") as ops, \
         tc.tile_pool(name="osb", bufs=2) as osb:
        pso = [ops.tile([P, SQ], f32, tag="ops", name=f"pso{i}") for i in range(8)]
        for j in range(8):
            wo_sb = wop.tile([P, D], f32r, tag="wo", name=f"wo{j}")
            nc.gpsimd.dma_start(out=wo_sb, in_=woT_r[j])
            for st in range(4):
                for nt in range(2):
                    nc.tensor.matmul(
                        pso[st * 2 + nt],
                        lhsT=attnT_sb[:, j, st * P:(st + 1) * P],
                        rhs=wo_sb[:, nt * SQ:(nt + 1) * SQ],
                        start=(j == 0), stop=(j == 7))
        for st in range(4):
            ob = osb.tile([P, D], f32, tag="ob", name=f"ob{st}")
            nc.vector.tensor_copy(out=ob[:, 0:SQ], in_=pso[st * 2])
            nc.vector.tensor_copy(out=ob[:, SQ:D], in_=pso[st * 2 + 1])
            nc.sync.dma_start(out=out_r[st], in_=ob)
    persist.release()


def _build():
    if "nc" in _CACHE:
        return _CACHE["nc"]
    nc = bacc.Bacc("TRN2", target_bir_lowering=False, debug=False)
    xT = nc.dram_tensor("xT", [D, S], f32, kind="ExternalInput").ap()
    wqT = nc.dram_tensor("wqT", [D, H * HD], f32, kind="ExternalInput").ap()
    wkT = nc.dram_tensor("wkT", [D, KVH * HD], f32, kind="ExternalInput").ap()
    wvT = nc.dram_tensor("wvT", [D, KVH * HD], f32, kind="ExternalInput").ap()
    woT = nc.dram_tensor("woT", [H * HD, D], f32, kind="ExternalInput").ap()
    mT = nc.dram_tensor("mT", [2, S, SQ], f32, kind="ExternalInput").ap()
    esel = nc.dram_tensor("esel", [2 * 8, 8, P], f32, kind="ExternalInput").ap()
    out = nc.dram_tensor("out", [SQ, D], f32, kind="ExternalOutput").ap()
    with tile.TileContext(nc) as tc:
        _body(tc, xT, wqT, wkT, wvT, woT, mT, esel, out)
    nc.compile()
    _CACHE["nc"] = nc
    return nc


def _host_prep(hidden_states, full_mask, tag_mask, wq, wk, wv, wo):
    # pair-ordered feature permutation for wq columns / wo.T rows
    perm = np.concatenate([np.r_[a * HD:(a + 1) * HD, b * HD:(b + 1) * HD]
                           for a, b in PAIRS])
    wqT = np.ascontiguousarray(wq.T[:, perm], np.float32)      # [D, 1024]
    wkT = np.ascontiguousarray(wk.T, np.float32)               # [D, 256]
    wvT = np.ascontiguousarray(wv.T, np.float32)               # [D, 256]
    woT = np.ascontiguousarray(wo.T[perm, :], np.float32)      # [1024, D]
    # broadcast selector: brc[p, q] = rr[2j + (p >= 64), q]
    esel = np.zeros((2 * 8, 8, P), np.float32)
    for j in range(8):
        esel[2 * j, j, 0:HD] = 1.0
        esel[2 * j + 1, j, HD:P] = 1.0
    masksT = [np.ascontiguousarray(full_mask[b, 0].T) for b in range(B)] + \
             [np.ascontiguousarray(tag_mask[b, 0].T) for b in range(B)]
    xTs = [np.ascontiguousarray(hidden_states[b].T, np.float32) for b in range(B)]
    in_maps = []
    for c in range(NCORES):
        b, q0 = c // 4, (c % 4) * SQ
        xT_c = np.roll(xTs[b], -q0, axis=1)
        fmT = np.roll(masksT[b][:, q0:q0 + SQ], -q0, axis=0)
        tgT = np.roll(masksT[2 + b][:, q0:q0 + SQ], -q0, axis=0)
        mT_c = np.ascontiguousarray(np.stack([fmT, tgT]), np.float32)
        in_maps.append({"xT": np.ascontiguousarray(xT_c), "wqT": wqT, "wkT": wkT,
                        "wvT": wvT, "woT": woT, "mT": mT_c, "esel": esel})
    return in_maps


def kernel(hidden_states, full_mask, tag_mask, wq, wk, wv, wo, _trace=False):
    args = [np.asarray(a, np.float32) for a in
            (hidden_states, full_mask, tag_mask, wq, wk, wv, wo)]
    nc = _build()
    in_maps = _host_prep(*args)
    try:
        res = run_bass_kernel_spmd(nc, in_maps, core_ids=list(range(NCORES)),
                                   trace=_trace)
    except ModuleNotFoundError:
        res = run_bass_kernel_spmd(nc, in_maps, core_ids=list(range(NCORES)))
    _CACHE["last_results"] = res
    full = np.empty((B, S, D), np.float32)
    for c in range(NCORES):
        b, q0 = c // 4, (c % 4) * SQ
        full[b, q0:q0 + SQ, :] = res.results[c]["out"]
    return full


# revision 15
# speedup vs baseline: 1.1450x; 1.1450x over previous
"""Trainium2 Bass kernel for nn_JanusModel (sparse_attention, GQA, two mask groups).

Sharding: core c in [0,8) handles batch b=c//4 and query-row block q0=(c%4)*512.
Each core computes all 16 heads for its 512 query rows -> disjoint output slices,
no collectives. All heavy operands are laid out on host (transposes/permutes only).

On-device math per core (scores kept transposed [sk, sq]):
  qT/kT/v projections (fp32r matmuls, q/k cast to bf16), scores.T = K @ qT/8
  (row-tiled head pairs, bf16), P = exp(scores) * exp(maskT) (ACT exp + DVE
  bf16 mul), AV col-tiled head pairs with a ones-column appended to V so row 64
  of each AV psum accumulates the softmax denominator for free. One batched
  reciprocal over all 16 rowsum rows, broadcast to 128 partitions via a tiny
  selector matmul, divide, then the output projection.
AV matmuls for group g are emitted after the score matmuls of group g+1 so the
PE never stalls behind the ACT->DVE chain of its own group.
"""

import os
import sys

import numpy as np

try:
    from ml_dtypes import bfloat16 as np_bf16
except ImportError:  # jax always ships ml_dtypes
    import jax.numpy as _jnp

    np_bf16 = _jnp.bfloat16

for _p in ("/opt/trn_rl_repo",):
    if os.path.isdir(_p) and _p not in sys.path:
        sys.path.insert(0, _p)

import concourse.bass as bass
import concourse.tile as tile
from concourse import bacc, mybir
from concourse.bass_utils import run_bass_kernel_spmd

B, S, D = 2, 2048, 1024
H, KVH, HD = 16, 4, 64
NCORES = 8
SQ = S // 4  # 512 query rows per core
P = 128
NKT = S // P  # 16 key tiles

# Head pairs: (a, b) share a kT tile; a uses kv head 2*(j//4), b uses +1.
PAIRS = [(0, 4), (1, 5), (2, 6), (3, 7), (8, 12), (9, 13), (10, 14), (11, 15)]

f32 = mybir.dt.float32
bf16 = mybir.dt.bfloat16
f32r = mybir.dt.float32r
EXP = mybir.ActivationFunctionType.Exp

_CACHE = {}


def _r(ap):
    return ap.bitcast(f32r)


def _body(tc, xT, wqT, wkT, wvT, woT, mT, esel, out):
    nc = tc.nc
    xT_r = xT.rearrange("(c p) s -> c p s", p=P)        # [8,128,2048]
    wqT_r = wqT.rearrange("(c p) f -> c p f", p=P)      # [8,128,1024]
    wkT_r = wkT.rearrange("(c p) f -> c p f", p=P)      # [8,128,256]
    wvT_r = wvT.rearrange("(c p) f -> c p f", p=P)      # [8,128,256]
    woT_r = woT.rearrange("(c p) d -> c p d", p=P)      # [8,128,1024]
    mT_r = mT.rearrange("m (c p) q -> m p c q", p=P)    # [2,128,16,512]
    out_r = out.rearrange("(t p) d -> t p d", p=P)      # [4,128,1024]

    persist = tc.alloc_tile_pool(name="persist", bufs=1)
    qT_sb = persist.tile([P, 8, SQ], bf16, name="qT_sb")      # pair j: a rows 0:64, b rows 64:128
    kT_sb = persist.tile([P, 2, S], bf16, name="kT_sb")       # tile jt: kv 2jt rows 0:64, kv 2jt+1 rows 64:128
    v_sb = persist.tile([P, NKT, KVH, HD + 1], bf16, name="v_sb")  # col 64 of each kv head = 1.0
    expm_sb = persist.tile([P, 2, NKT, SQ], bf16, name="expm_sb")
    attnT_sb = persist.tile([P, 8, SQ], bf16, name="attnT_sb")
    # rowsum staging: DVE writes must start at partition 0/32/64/96, so the 16
    # denominator rows (index r = 2j+half) land at partition 32*(r//4), column
    # slot r%4; one SBUF->SBUF scatter DMA later packs them to 16 partitions.
    rstage_sb = persist.tile([P, 4, SQ], f32, name="rstage_sb")

    nc.vector.memset(v_sb[:, :, :, HD:HD + 1], 1.0)

    # ---------------- phase A: masks exp + load x/w + projections ----------------
    with tc.tile_pool(name="ml", bufs=2) as mlp, \
         tc.tile_pool(name="xw", bufs=1) as xw, \
         tc.tile_pool(name="pps", bufs=4, space="PSUM") as pps:
        x_sb = xw.tile([P, 8, S], bf16, name="x_sb")
        wq_sb = xw.tile([P, 8, H * HD], bf16, name="wq_sb")
        wk_sb = xw.tile([P, 8, KVH * HD], bf16, name="wk_sb")
        wv_sb = xw.tile([P, 8, KVH * HD], bf16, name="wv_sb")
        # even x chunks ride the sync queue (ahead of the masks), odd chunks
        # and the weights ride gpsimd: the first q-proj matmul only needs
        # x0+wq0, so it fires a few us in.
        for c in range(0, 8, 2):
            nc.sync.dma_start(out=x_sb[:, c, :], in_=xT_r[c])
        for c in range(8):
            if c % 2:
                nc.gpsimd.dma_start(out=x_sb[:, c, :], in_=xT_r[c])
            nc.gpsimd.dma_start(out=wq_sb[:, c, :], in_=wqT_r[c])
        for c in range(8):
            nc.gpsimd.dma_start(out=wk_sb[:, c, :], in_=wkT_r[c])
        for c in range(8):
            nc.gpsimd.dma_start(out=wv_sb[:, c, :], in_=wvT_r[c])

        # mask DMAs follow the x chunks on the sync queue; exps fill the ACT
        # engine while the projections stream on the PE.  bufs=2 bounds the
        # prefetch depth so mask traffic doesn't starve the x/w loads.
        for m in range(2):
            for tg in range(8):
                ml = mlp.tile([P, 2, SQ], f32, tag="ml", name=f"ml{m}{tg}")
                nc.sync.dma_start(out=ml, in_=mT_r[m, :, 2 * tg:2 * tg + 2, :])
                nc.scalar.activation(
                    out=expm_sb[:, m, 2 * tg:2 * tg + 2, :], in_=ml, func=EXP)

        def qproj(j):
            # out [128 qfeat(pair j), 512]; fold 1/sqrt(HD)=1/8 scale, cast bf16
            ps = pps.tile([P, SQ], f32, tag="pq", name=f"psq{j}")
            for kc in range(8):
                nc.tensor.matmul(
                    ps, lhsT=wq_sb[:, kc, j * P:(j + 1) * P],
                    rhs=x_sb[:, kc, 0:SQ], start=(kc == 0), stop=(kc == 7))
            nc.vector.tensor_scalar_mul(qT_sb[:, j, :], ps, 0.125)

        def kproj(jt):
            for ns in range(4):
                ps = pps.tile([P, SQ], f32, tag="pq", name=f"psk{jt}{ns}")
                for kc in range(8):
                    nc.tensor.matmul(
                        ps, lhsT=wk_sb[:, kc, jt * P:(jt + 1) * P],
                        rhs=x_sb[:, kc, ns * SQ:(ns + 1) * SQ],
                        start=(kc == 0), stop=(kc == 7))
                nc.vector.tensor_copy(out=kT_sb[:, jt, ns * SQ:(ns + 1) * SQ], in_=ps)

        qproj(0)
        kproj(0)  # pair 0 consumes qT[0] + kT tile 0 first
        for j in range(1, 8):
            qproj(j)
        kproj(1)

        # v projection: natural [sk 128-tile, 4, 64] -> bf16, 65-strided slots
        for t in range(NKT):
            ps = pps.tile([P, KVH * HD], f32, tag="pv", name=f"psv{t}")
            for kc in range(8):
                nc.tensor.matmul(
                    ps, lhsT=x_sb[:, kc, t * P:(t + 1) * P],
                    rhs=wv_sb[:, kc, :], start=(kc == 0), stop=(kc == 7))
            nc.vector.tensor_copy(
                out=v_sb[:, t, :, 0:HD], in_=ps.rearrange("p (h c) -> p h c", h=KVH))

    # ---------------- phase B: attention ----------------
    with tc.tile_pool(name="psA", bufs=1, space="PSUM") as psA, \
         tc.tile_pool(name="psB", bufs=1, space="PSUM") as psB, \
         tc.tile_pool(name="avp", bufs=2, space="PSUM") as avp, \
         tc.tile_pool(name="praw", bufs=2) as praw, \
         tc.tile_pool(name="ppool", bufs=3) as ppool:
        GROUPS = [(g, 2) for g in range(NKT // 2)]
        LASTG = GROUPS[-1][0]

        def emit_av(j, g, nt, pA, pB, av_a, av_b, last):
            kv = (j // 4) * 2
            for i in range(nt):
                t = 2 * g + i
                st = (t == 0)
                sp = (t == NKT - 1)
                nc.tensor.matmul(av_a[0:HD + 1, :], lhsT=v_sb[:, t, kv, :],
                                 rhs=pA[:, i, :], start=st, stop=sp)
                nc.tensor.matmul(av_b[0:HD + 1, :], lhsT=v_sb[:, t, kv + 1, :],
                                 rhs=pB[:, i, :], start=st, stop=sp)
            if last:
                nc.vector.tensor_copy(out=attnT_sb[0:HD, j, :], in_=av_a[0:HD, :])
                nc.vector.tensor_copy(out=attnT_sb[HD:P, j, :], in_=av_b[0:HD, :])
                for half, av in ((0, av_a), (1, av_b)):
                    r = 2 * j + half
                    p0 = 32 * (r // 4)
                    nc.vector.tensor_copy(out=rstage_sb[p0:p0 + 1, r % 4, :],
                                          in_=av[HD:HD + 1, :])

        pending = None
        for j in range(8):
            jt = j // 4
            m = j // 4
            av_a = avp.tile([P, SQ], f32, tag="ava", name=f"ava{j}")
            av_b = avp.tile([P, SQ], f32, tag="avb", name=f"avb{j}")
            for g, nt in GROUPS:
                sA = psA.tile([P, 2, SQ], f32, tag="sA", name=f"sA{j}_{g}")
                sB = psB.tile([P, 2, SQ], f32, tag="sB", name=f"sB{j}_{g}")
                for i in range(nt):
                    t = 2 * g + i
                    nc.tensor.matmul(
                        sA[:, i, :], lhsT=kT_sb[0:HD, jt, t * P:(t + 1) * P],
                        rhs=qT_sb[0:HD, j, :], start=True, stop=True)
                    nc.tensor.matmul(
                        sB[:, i, :], lhsT=kT_sb[HD:P, jt, t * P:(t + 1) * P],
                        rhs=qT_sb[HD:P, j, :], start=True, stop=True)
                if pending is not None:
                    emit_av(*pending)
                    pending = None
                prA = praw.tile([P, 2, SQ], bf16, tag="prA", name=f"prA{j}_{g}")
                prB = praw.tile([P, 2, SQ], bf16, tag="prB", name=f"prB{j}_{g}")
                nc.scalar.activation(out=prA[:, 0:nt, :], in_=sA[:, 0:nt, :], func=EXP)
                nc.scalar.activation(out=prB[:, 0:nt, :], in_=sB[:, 0:nt, :], func=EXP)
                pA = ppool.tile([P, 2, SQ], bf16, tag="pA", name=f"pA{j}_{g}")
                pB = ppool.tile([P, 2, SQ], bf16, tag="pB", name=f"pB{j}_{g}")
                nc.vector.tensor_mul(pA[:, 0:nt, :], prA[:, 0:nt, :],
                                     expm_sb[:, m, 2 * g:2 * g + nt, :])
                nc.vector.tensor_mul(pB[:, 0:nt, :], prB[:, 0:nt, :],
                                     expm_sb[:, m, 2 * g:2 * g + nt, :])
                pending = (j, g, nt, pA, pB, av_a, av_b, g == LASTG)
        emit_av(*pending)

    # softmax denominators: one batched reciprocal, PE broadcast, divide
    with tc.tile_pool(name="divp", bufs=1) as divp, \
         tc.tile_pool(name="brcp", bufs=2, space="PSUM") as brcp:
        rs_sb = divp.tile([2 * 8, SQ], f32, name="rs_sb")
        rr_sb = divp.tile([2 * 8, SQ], f32r, name="rr_sb")
        esel_sb = divp.tile([2 * 8, 8, P], f32r, name="esel_sb")
        nc.gpsimd.dma_start(out=esel_sb, in_=esel)
        for s in range(4):
            nc.sync.dma_start(out=rs_sb[4 * s:4 * s + 4, :],
                              in_=rstage_sb[32 * s:32 * s + 1, :, :])
        with nc.allow_low_precision(reason="fp32r denominators feed the PE broadcast"):
            nc.vector.reciprocal(out=rr_sb, in_=rs_sb)
        for j in range(8):
            brc = brcp.tile([P, SQ], f32, tag="brc", name=f"brc{j}")
            nc.tensor.matmul(brc, lhsT=esel_sb[:, j, :], rhs=rr_sb,
                             start=True, stop=True)
            nc.vector.tensor_mul(attnT_sb[:, j, :], attnT_sb[:, j, :], brc)

    # ---------------- phase C: output projection ----------------
    with tc.tile_pool(name="wo", bufs=3) as wop, \
         tc.tile_pool(name="ops", bufs=8, space="PSUM") as ops, \
         tc.tile_pool(name="osb", bufs=2) as osb:
        pso = [ops.tile([P, SQ], f32, tag="ops", name=f"pso{i}") for i in range(8)]
        for j in range(8):
            wo_sb = wop.tile([P, D], bf16, tag="wo", name=f"wo{j}")
            nc.sync.dma_start(out=wo_sb, in_=woT_r[j])
            for st in range(4):
                for nt in range(2):
                    nc.tensor.matmul(
                        pso[st * 2 + nt],
                        lhsT=attnT_sb[:, j, st * P:(st + 1) * P],
                        rhs=wo_sb[:, nt * SQ:(nt + 1) * SQ],
                        start=(j == 0), stop=(j == 7))
        for st in range(4):
            ob = osb.tile([P, D], f32, tag="ob", name=f"ob{st}")
            nc.vector.tensor_copy(out=ob[:, 0:SQ], in_=pso[st * 2])
            nc.vector.tensor_copy(out=ob[:, SQ:D], in_=pso[st * 2 + 1])
            nc.sync.dma_start(out=out_r[st], in_=ob)
    persist.release()


def _build():
    if "nc" in _CACHE:
        return _CACHE["nc"]
    nc = bacc.Bacc("TRN2", target_bir_lowering=False, debug=False)
    xT = nc.dram_tensor("xT", [D, S], bf16, kind="ExternalInput").ap()
    wqT = nc.dram_tensor("wqT", [D, H * HD], bf16, kind="ExternalInput").ap()
    wkT = nc.dram_tensor("wkT", [D, KVH * HD], bf16, kind="ExternalInput").ap()
    wvT = nc.dram_tensor("wvT", [D, KVH * HD], bf16, kind="ExternalInput").ap()
    woT = nc.dram_tensor("woT", [H * HD, D], bf16, kind="ExternalInput").ap()
    mT = nc.dram_tensor("mT", [2, S, SQ], f32, kind="ExternalInput").ap()
    esel = nc.dram_tensor("esel", [2 * 8, 8, P], f32, kind="ExternalInput").ap()
    out = nc.dram_tensor("out", [SQ, D], f32, kind="ExternalOutput").ap()
    with tile.TileContext(nc) as tc:
        _body(tc, xT, wqT, wkT, wvT, woT, mT, esel, out)
    nc.compile()
    _CACHE["nc"] = nc
    return nc


def _host_prep(hidden_states, full_mask, tag_mask, wq, wk, wv, wo):
    # pair-ordered feature permutation for wq columns / wo.T rows
    perm = np.concatenate([np.r_[a * HD:(a + 1) * HD, b * HD:(b + 1) * HD]
                           for a, b in PAIRS])
    wqT = np.ascontiguousarray(wq.T[:, perm].astype(np_bf16))  # [D, 1024]
    wkT = np.ascontiguousarray(wk.T.astype(np_bf16))           # [D, 256]
    wvT = np.ascontiguousarray(wv.T.astype(np_bf16))           # [D, 256]
    woT = np.ascontiguousarray(wo.T[perm, :].astype(np_bf16))  # [1024, D]
    # broadcast selector: brc[p, q] = rr[2j + (p >= 64), q]
    esel = np.zeros((2 * 8, 8, P), np.float32)
    for j in range(8):
        esel[2 * j, j, 0:HD] = 1.0
        esel[2 * j + 1, j, HD:P] = 1.0
    masksT = [np.ascontiguousarray(full_mask[b, 0].T) for b in range(B)] + \
             [np.ascontiguousarray(tag_mask[b, 0].T) for b in range(B)]
    xTs = [np.ascontiguousarray(hidden_states[b].T.astype(np_bf16)) for b in range(B)]
    in_maps = []
    for c in range(NCORES):
        b, q0 = c // 4, (c % 4) * SQ
        xT_c = np.roll(xTs[b], -q0, axis=1)
        fmT = np.roll(masksT[b][:, q0:q0 + SQ], -q0, axis=0)
        tgT = np.roll(masksT[2 + b][:, q0:q0 + SQ], -q0, axis=0)
        mT_c = np.ascontiguousarray(np.stack([fmT, tgT]), np.float32)
        in_maps.append({"xT": np.ascontiguousarray(xT_c), "wqT": wqT, "wkT": wkT,
                        "wvT": wvT, "woT": woT, "mT": mT_c, "esel": esel})
    return in_maps


def kernel(hidden_states, full_mask, tag_mask, wq, wk, wv, wo, _trace=False):
    args = [np.asarray(a, np.float32) for a in
            (hidden_states, full_mask, tag_mask, wq, wk, wv, wo)]
    nc = _build()
    in_maps = _host_prep(*args)
    try:
        res = run_bass_kernel_spmd(nc, in_maps, core_ids=list(range(NCORES)),
                                   trace=_trace)
    except ModuleNotFoundError:
        res = run_bass_kernel_spmd(nc, in_maps, core_ids=list(range(NCORES)))
    _CACHE["last_results"] = res
    full = np.empty((B, S, D), np.float32)
    for c in range(NCORES):
        b, q0 = c // 4, (c % 4) * SQ
        full[b, q0:q0 + SQ, :] = res.results[c]["out"]
    return full


# revision 18
# speedup vs baseline: 1.2057x; 1.0530x over previous
"""Trainium2 Bass kernel for nn_JanusModel (sparse_attention, GQA, two mask groups).

Sharding: core c in [0,8) handles batch b=c//4 and query-row block q0=(c%4)*512.
Each core computes all 16 heads for its 512 query rows -> disjoint output slices,
no collectives. Heavy operands are transposed/permuted/bf16-cast on host.

Per core (scores kept transposed [sk, sq], all matmul operands bf16):
  qT/kT/v projections, scoresT = K @ qT/8 for head pairs, P = exp(scores) *
  exp(maskT), AV with a ones-column in V so row 64 of each AV psum accumulates
  the softmax denominator for free, batched reciprocal + selector-matmul
  broadcast for the division, then the output projection.

Pipeline: one merged [128,4,512] score psum per 2-key-tile group feeds a single
ACT exp; AV matmuls lag two groups behind scores; mask exps are emitted
just-in-time inside pairs 0/4 so the in-order ACT queue never blocks score
exps on mask DMAs; leftover phase-A projection matmuls are interleaved into
pairs 0-1 as PE filler so the tensor engine never idles (keeps the HAM power
state at full throughput).
"""

import os
import sys

import numpy as np

try:
    from ml_dtypes import bfloat16 as np_bf16
except ImportError:  # jax always ships ml_dtypes
    import jax.numpy as _jnp

    np_bf16 = _jnp.bfloat16

for _p in ("/opt/trn_rl_repo",):
    if os.path.isdir(_p) and _p not in sys.path:
        sys.path.insert(0, _p)

import concourse.bass as bass
import concourse.tile as tile
from concourse import bacc, mybir
from concourse.bass_utils import run_bass_kernel_spmd

B, S, D = 2, 2048, 1024
H, KVH, HD = 16, 4, 64
NCORES = 8
SQ = S // 4  # 512 query rows per core
P = 128
NKT = S // P  # 16 key tiles
NG = NKT // 2  # 8 groups of 2 key tiles

# Head pairs: (a, b) share a kT tile; a uses kv head 2*(j//4), b uses +1.
PAIRS = [(0, 4), (1, 5), (2, 6), (3, 7), (8, 12), (9, 13), (10, 14), (11, 15)]

f32 = mybir.dt.float32
bf16 = mybir.dt.bfloat16
f32r = mybir.dt.float32r
EXP = mybir.ActivationFunctionType.Exp

_CACHE = {}


def _r(ap):
    return ap.bitcast(f32r)


def _body(tc, xT, wqT, wkT, wvT, woT, mT, esel, out):
    nc = tc.nc
    xT_r = xT.rearrange("(c p) s -> c p s", p=P)        # [8,128,2048]
    wqT_r = wqT.rearrange("(c p) f -> c p f", p=P)      # [8,128,1024]
    wkT_r = wkT.rearrange("(c p) f -> c p f", p=P)      # [8,128,256]
    wvT_r = wvT.rearrange("(c p) f -> c p f", p=P)      # [8,128,256]
    woT_r = woT.rearrange("(c p) d -> c p d", p=P)      # [8,128,1024]
    mT_r = mT.rearrange("m (c p) q -> m p c q", p=P)    # [2,128,16,512]
    out_r = out.rearrange("(t p) d -> t p d", p=P)      # [4,128,1024]

    persist = tc.alloc_tile_pool(name="persist", bufs=1)
    qT_sb = persist.tile([P, 8, SQ], bf16, name="qT_sb")      # pair j: a rows 0:64, b 64:128
    kT_sb = persist.tile([P, 2, S], bf16, name="kT_sb")       # tile jt: kv 2jt rows 0:64, +1 rows 64:128
    v_sb = persist.tile([P, NKT, KVH, HD + 1], bf16, name="v_sb")  # col 64 of each kv head = 1.0
    expm_sb = persist.tile([P, 2, NKT, SQ], bf16, name="expm_sb")
    attnT_sb = persist.tile([P, 8, SQ], bf16, name="attnT_sb")
    # rowsum staging: DVE writes must start at partition 0/32/64/96, so the 16
    # denominator rows (index r = 2j+half) land at partition 32*(r//4), column
    # slot r%4; SBUF->SBUF scatter DMAs later pack them to 16 partitions.
    rstage_sb = persist.tile([P, 4, SQ], f32, name="rstage_sb")

    nc.vector.memset(v_sb[:, :, :, HD:HD + 1], 1.0)

    # mask staging + probability pools outlive the xw pool region, so allocate
    # them first: a later-allocated pool reusing xw's bytes would chain phase-B
    # ACT/DVE work behind the last phase-A matmul.
    mlp = tc.alloc_tile_pool(name="mlp", bufs=6)
    praw = tc.alloc_tile_pool(name="praw", bufs=2)
    ppool = tc.alloc_tile_pool(name="ppool", bufs=4)

    with tc.tile_pool(name="xw", bufs=1) as xw, \
         tc.tile_pool(name="psS", bufs=1, space="PSUM") as psS, \
         tc.tile_pool(name="avp", bufs=1, space="PSUM") as avp, \
         tc.tile_pool(name="ppj", bufs=1, space="PSUM") as ppj:
        x_sb = xw.tile([P, 8, S], bf16, name="x_sb")
        wq_sb = xw.tile([P, 8, H * HD], bf16, name="wq_sb")
        wk_sb = xw.tile([P, 8, KVH * HD], bf16, name="wk_sb")
        wv_sb = xw.tile([P, 8, KVH * HD], bf16, name="wv_sb")
        # even x chunks ride the sync queue (ahead of the masks), odd chunks
        # and the weights ride gpsimd: the first q-proj matmul only needs
        # x0 + wq0, so the PE starts a few us in.
        for c in range(0, 8, 2):
            nc.sync.dma_start(out=x_sb[:, c, :], in_=xT_r[c])
        for c in range(8):
            if c % 2:
                nc.gpsimd.dma_start(out=x_sb[:, c, :], in_=xT_r[c])
            nc.gpsimd.dma_start(out=wq_sb[:, c, :], in_=wqT_r[c])
        for c in range(8):
            nc.gpsimd.dma_start(out=wk_sb[:, c, :], in_=wkT_r[c])
        for c in range(8):
            nc.gpsimd.dma_start(out=wv_sb[:, c, :], in_=wvT_r[c])

        # mask DMAs queue behind the even x chunks on sync; issue is paced by
        # the mlp pool so each tile lands just ahead of its phase-B exp.
        ml_tiles = {}
        for m in range(2):
            for g in range(NG):
                ml = mlp.tile([P, 2, SQ], f32, tag="ml", name=f"ml{m}{g}")
                nc.sync.dma_start(out=ml, in_=mT_r[m, :, 2 * g:2 * g + 2, :])
                ml_tiles[(m, g)] = ml

        def qproj(j):
            # out [128 qfeat(pair j), 512]; fold 1/sqrt(HD)=1/8, cast bf16
            ps = ppj.tile([P, SQ], f32, tag="pq", name=f"psq{j}")
            for kc in range(8):
                nc.tensor.matmul(
                    ps, lhsT=wq_sb[:, kc, j * P:(j + 1) * P],
                    rhs=x_sb[:, kc, 0:SQ], start=(kc == 0), stop=(kc == 7))
            nc.vector.tensor_scalar_mul(qT_sb[:, j, :], ps, 0.125)

        def kproj(jt, ns):
            ps = ppj.tile([P, SQ], f32, tag="pq", name=f"psk{jt}{ns}")
            for kc in range(8):
                nc.tensor.matmul(
                    ps, lhsT=wk_sb[:, kc, jt * P:(jt + 1) * P],
                    rhs=x_sb[:, kc, ns * SQ:(ns + 1) * SQ],
                    start=(kc == 0), stop=(kc == 7))
            nc.vector.tensor_copy(out=kT_sb[:, jt, ns * SQ:(ns + 1) * SQ], in_=ps)

        def vproj(t):
            ps = ppj.tile([P, KVH * HD], f32, tag="pv", name=f"psv{t}")
            for kc in range(8):
                nc.tensor.matmul(
                    ps, lhsT=x_sb[:, kc, t * P:(t + 1) * P],
                    rhs=wv_sb[:, kc, :], start=(kc == 0), stop=(kc == 7))
            nc.vector.tensor_copy(
                out=v_sb[:, t, :, 0:HD], in_=ps.rearrange("p (h c) -> p h c", h=KVH))

        # head start: everything pair 0 group 0 needs
        qproj(0)
        for ns in range(4):
            kproj(0, ns)
        vproj(0)
        vproj(1)

        # remaining phase-A work, doled out as PE filler inside pairs 0-1;
        # vproj(t) stays >= 2 tiles ahead of the AV consumer (which itself
        # lags scores by 2 groups).
        filler = []
        for g in range(7):
            filler.append([lambda t=2 * g + 2: vproj(t),
                           lambda t=2 * g + 3: vproj(t),
                           lambda j=g + 1: qproj(j)])
        filler.append([lambda: kproj(1, 0), lambda: kproj(1, 1)])
        filler.append([lambda: kproj(1, 2), lambda: kproj(1, 3)])

        # ---------------- phase B: attention ----------------
        def emit_av(j, g, pA, pB, av_a, av_b):
            kv = (j // 4) * 2
            for i in range(2):
                t = 2 * g + i
                st = (t == 0)
                sp = (t == NKT - 1)
                nc.tensor.matmul(av_a[0:HD + 1, :], lhsT=v_sb[:, t, kv, :],
                                 rhs=pA[:, i, :], start=st, stop=sp)
                nc.tensor.matmul(av_b[0:HD + 1, :], lhsT=v_sb[:, t, kv + 1, :],
                                 rhs=pB[:, i, :], start=st, stop=sp)
            if g == NG - 1:
                nc.vector.tensor_copy(out=attnT_sb[0:HD, j, :], in_=av_a[0:HD, :])
                nc.vector.tensor_copy(out=attnT_sb[HD:P, j, :], in_=av_b[0:HD, :])
                for half, av in ((0, av_a), (1, av_b)):
                    r = 2 * j + half
                    nc.vector.tensor_copy(
                        out=rstage_sb[32 * (r // 4):32 * (r // 4) + 1, r % 4, :],
                        in_=av[HD:HD + 1, :])

        pending = []
        fidx = 0
        for j in range(8):
            jt = j // 4
            m = j // 4
            av_a = avp.tile([P, SQ], f32, tag="ava", name=f"ava{j}")
            av_b = avp.tile([P, SQ], f32, tag="avb", name=f"avb{j}")
            for g in range(NG):
                # just-in-time mask exp for the mask this half of the pairs uses
                if j in (0, 4):
                    nc.scalar.activation(out=expm_sb[:, m, 2 * g:2 * g + 2, :],
                                         in_=ml_tiles[(m, g)], func=EXP)
                sS = psS.tile([P, 4, SQ], f32, tag="s", name=f"s{j}_{g}")
                for i in range(2):
                    t = 2 * g + i
                    nc.tensor.matmul(
                        sS[:, i, :], lhsT=kT_sb[0:HD, jt, t * P:(t + 1) * P],
                        rhs=qT_sb[0:HD, j, :], start=True, stop=True)
                    nc.tensor.matmul(
                        sS[:, 2 + i, :], lhsT=kT_sb[HD:P, jt, t * P:(t + 1) * P],
                        rhs=qT_sb[HD:P, j, :], start=True, stop=True)
                if len(pending) >= 2:
                    emit_av(*pending.pop(0))
                if fidx < len(filler):
                    for fn in filler[fidx]:
                        fn()
                    fidx += 1
                pr = praw.tile([P, 4, SQ], bf16, tag="pr", name=f"pr{j}_{g}")
                nc.scalar.activation(out=pr, in_=sS, func=EXP)
                pA = ppool.tile([P, 2, SQ], bf16, tag="pA", name=f"pA{j}_{g}")
                pB = ppool.tile([P, 2, SQ], bf16, tag="pB", name=f"pB{j}_{g}")
                nc.vector.tensor_mul(pA, pr[:, 0:2, :],
                                     expm_sb[:, m, 2 * g:2 * g + 2, :])
                nc.vector.tensor_mul(pB, pr[:, 2:4, :],
                                     expm_sb[:, m, 2 * g:2 * g + 2, :])
                pending.append((j, g, pA, pB, av_a, av_b))
        while pending:
            emit_av(*pending.pop(0))

    ppool.release()
    praw.release()
    mlp.release()

    # softmax denominators: one batched reciprocal, PE broadcast, divide
    with tc.tile_pool(name="divp", bufs=1) as divp, \
         tc.tile_pool(name="brcp", bufs=2, space="PSUM") as brcp:
        rs_sb = divp.tile([2 * 8, SQ], f32, name="rs_sb")
        rr_sb = divp.tile([2 * 8, SQ], f32r, name="rr_sb")
        esel_sb = divp.tile([2 * 8, 8, P], f32r, name="esel_sb")
        nc.gpsimd.dma_start(out=esel_sb, in_=esel)
        for s in range(4):
            nc.sync.dma_start(out=rs_sb[4 * s:4 * s + 4, :],
                              in_=rstage_sb[32 * s:32 * s + 1, :, :])
        with nc.allow_low_precision(reason="fp32r denominators feed the PE broadcast"):
            nc.vector.reciprocal(out=rr_sb, in_=rs_sb)
        for j in range(8):
            brc = brcp.tile([P, SQ], f32, tag="brc", name=f"brc{j}")
            nc.tensor.matmul(brc, lhsT=esel_sb[:, j, :], rhs=rr_sb,
                             start=True, stop=True)
            nc.vector.tensor_mul(attnT_sb[:, j, :], attnT_sb[:, j, :], brc)

    # ---------------- phase C: output projection ----------------
    with tc.tile_pool(name="wo", bufs=8) as wop, \
         tc.tile_pool(name="ops", bufs=8, space="PSUM") as ops, \
         tc.tile_pool(name="osb", bufs=2) as osb:
        pso = [ops.tile([P, SQ], f32, tag="ops", name=f"pso{i}") for i in range(8)]
        for j in range(8):
            wo_sb = wop.tile([P, D], bf16, tag="wo", name=f"wo{j}")
            nc.gpsimd.dma_start(out=wo_sb, in_=woT_r[j])
            for st in range(4):
                for nt in range(2):
                    nc.tensor.matmul(
                        pso[st * 2 + nt],
                        lhsT=attnT_sb[:, j, st * P:(st + 1) * P],
                        rhs=wo_sb[:, nt * SQ:(nt + 1) * SQ],
                        start=(j == 0), stop=(j == 7))
        for st in range(4):
            ob = osb.tile([P, D], f32, tag="ob", name=f"ob{st}")
            nc.vector.tensor_copy(out=ob[:, 0:SQ], in_=pso[st * 2])
            nc.vector.tensor_copy(out=ob[:, SQ:D], in_=pso[st * 2 + 1])
            nc.sync.dma_start(out=out_r[st], in_=ob)
    persist.release()


def _build():
    if "nc" in _CACHE:
        return _CACHE["nc"]
    nc = bacc.Bacc("TRN2", target_bir_lowering=False, debug=False)
    xT = nc.dram_tensor("xT", [D, S], bf16, kind="ExternalInput").ap()
    wqT = nc.dram_tensor("wqT", [D, H * HD], bf16, kind="ExternalInput").ap()
    wkT = nc.dram_tensor("wkT", [D, KVH * HD], bf16, kind="ExternalInput").ap()
    wvT = nc.dram_tensor("wvT", [D, KVH * HD], bf16, kind="ExternalInput").ap()
    woT = nc.dram_tensor("woT", [H * HD, D], bf16, kind="ExternalInput").ap()
    mT = nc.dram_tensor("mT", [2, S, SQ], f32, kind="ExternalInput").ap()
    esel = nc.dram_tensor("esel", [2 * 8, 8, P], f32, kind="ExternalInput").ap()
    out = nc.dram_tensor("out", [SQ, D], f32, kind="ExternalOutput").ap()
    with tile.TileContext(nc) as tc:
        _body(tc, xT, wqT, wkT, wvT, woT, mT, esel, out)
    nc.compile()
    _CACHE["nc"] = nc
    return nc


def _host_prep(hidden_states, full_mask, tag_mask, wq, wk, wv, wo):
    # pair-ordered feature permutation for wq columns / wo.T rows
    perm = np.concatenate([np.r_[a * HD:(a + 1) * HD, b * HD:(b + 1) * HD]
                           for a, b in PAIRS])
    wqT = np.ascontiguousarray(wq.T[:, perm].astype(np_bf16))  # [D, 1024]
    wkT = np.ascontiguousarray(wk.T.astype(np_bf16))           # [D, 256]
    wvT = np.ascontiguousarray(wv.T.astype(np_bf16))           # [D, 256]
    woT = np.ascontiguousarray(wo.T[perm, :].astype(np_bf16))  # [1024, D]
    # broadcast selector: brc[p, q] = rr[2j + (p >= 64), q]
    esel = np.zeros((2 * 8, 8, P), np.float32)
    for j in range(8):
        esel[2 * j, j, 0:HD] = 1.0
        esel[2 * j + 1, j, HD:P] = 1.0
    masksT = [np.ascontiguousarray(full_mask[b, 0].T) for b in range(B)] + \
             [np.ascontiguousarray(tag_mask[b, 0].T) for b in range(B)]
    xTs = [np.ascontiguousarray(hidden_states[b].T.astype(np_bf16)) for b in range(B)]
    in_maps = []
    for c in range(NCORES):
        b, q0 = c // 4, (c % 4) * SQ
        xT_c = np.roll(xTs[b], -q0, axis=1)
        fmT = np.roll(masksT[b][:, q0:q0 + SQ], -q0, axis=0)
        tgT = np.roll(masksT[2 + b][:, q0:q0 + SQ], -q0, axis=0)
        mT_c = np.ascontiguousarray(np.stack([fmT, tgT]), np.float32)
        in_maps.append({"xT": np.ascontiguousarray(xT_c), "wqT": wqT, "wkT": wkT,
                        "wvT": wvT, "woT": woT, "mT": mT_c, "esel": esel})
    return in_maps


def kernel(hidden_states, full_mask, tag_mask, wq, wk, wv, wo, _trace=False):
    args = [np.asarray(a, np.float32) for a in
            (hidden_states, full_mask, tag_mask, wq, wk, wv, wo)]
    nc = _build()
    in_maps = _host_prep(*args)
    try:
        res = run_bass_kernel_spmd(nc, in_maps, core_ids=list(range(NCORES)),
                                   trace=_trace)
    except ModuleNotFoundError:
        res = run_bass_kernel_spmd(nc, in_maps, core_ids=list(range(NCORES)))
    _CACHE["last_results"] = res
    full = np.empty((B, S, D), np.float32)
    for c in range(NCORES):
        b, q0 = c // 4, (c % 4) * SQ
        full[b, q0:q0 + SQ, :] = res.results[c]["out"]
    return full
